# revision 2
# baseline (speedup 1.0000x reference)
import numpy as np
import jax
import jax.numpy as jnp
from functools import partial

# nn_ConvolutionAutoEncoder: B=32, L=8192, C=64.
# Data parallel over batch across 8 NeuronCores (4 samples/core), weights
# replicated. The circular conv (rfft/irfft in the reference) is lowered to a
# four-step matmul DFT (8192 = 64 x 128) with explicit real/imag planes so the
# whole graph compiles to TensorE matmuls + elementwise ops on trn2.

C = 64
N_CORES = 8
L = 8192
N1, N2 = 64, 128  # t = n1*128 + n2 ; k = k1 + 64*k2


def _dft_consts():
    n1 = np.arange(N1)
    n2 = np.arange(N2)
    W64 = np.exp(-2j * np.pi * np.outer(n1, n1) / N1)        # [n1, k1]
    W128 = np.exp(-2j * np.pi * np.outer(n2, n2) / N2)       # [n2, k2]
    TW = np.exp(-2j * np.pi * np.outer(n1, n2) / L)          # [k1, n2]
    c = {}
    c['w64r'] = jnp.asarray(W64.real, jnp.float32)
    c['w64i'] = jnp.asarray(W64.imag, jnp.float32)
    c['w128r'] = jnp.asarray(W128.real, jnp.float32)
    c['w128i'] = jnp.asarray(W128.imag, jnp.float32)
    c['twr'] = jnp.asarray(TW.real, jnp.float32)
    c['twi'] = jnp.asarray(TW.imag, jnp.float32)
    # inverse (conjugate) matrices, 1/L folded into iw64
    c['iw128r'] = jnp.asarray(W128.real.T, jnp.float32)      # [k2, n2]
    c['iw128i'] = jnp.asarray(-W128.imag.T, jnp.float32)
    c['iw64r'] = jnp.asarray(W64.real.T / L, jnp.float32)    # [k1, n1]
    c['iw64i'] = jnp.asarray(-W64.imag.T / L, jnp.float32)
    return c


_DFT = _dft_consts()


def _fwd_fft_real(a):
    # a: [B, n1, n2, C] real -> (Xr, Xi): [B, k1, k2, C] at k = k1 + 64*k2
    br = jnp.einsum('bnmc,nk->bkmc', a, _DFT['w64r'])
    bi = jnp.einsum('bnmc,nk->bkmc', a, _DFT['w64i'])
    cr = br * _DFT['twr'][None, :, :, None] - bi * _DFT['twi'][None, :, :, None]
    ci = br * _DFT['twi'][None, :, :, None] + bi * _DFT['twr'][None, :, :, None]
    xr = jnp.einsum('bkmc,ml->bklc', cr, _DFT['w128r']) - \
         jnp.einsum('bkmc,ml->bklc', ci, _DFT['w128i'])
    xi = jnp.einsum('bkmc,ml->bklc', cr, _DFT['w128i']) + \
         jnp.einsum('bkmc,ml->bklc', ci, _DFT['w128r'])
    return xr, xi


def _inv_fft_real(pr, pi):
    # (pr, pi): [B, k1, k2, C] -> real [B, n1, n2, C]
    qr = jnp.einsum('bklc,lm->bkmc', pr, _DFT['iw128r']) - \
         jnp.einsum('bklc,lm->bkmc', pi, _DFT['iw128i'])
    qi = jnp.einsum('bklc,lm->bkmc', pr, _DFT['iw128i']) + \
         jnp.einsum('bklc,lm->bkmc', pi, _DFT['iw128r'])
    rr = qr * _DFT['twr'][None, :, :, None] + qi * _DFT['twi'][None, :, :, None]
    ri = -qr * _DFT['twi'][None, :, :, None] + qi * _DFT['twr'][None, :, :, None]
    out = jnp.einsum('bkmc,kn->bnmc', rr, _DFT['iw64r']) - \
          jnp.einsum('bkmc,kn->bnmc', ri, _DFT['iw64i'])
    return out


def _circ_conv_mm(x, f):
    # x: [B, L, C], f: [B, L//2, C] -> circular conv along time, per (b, c)
    B = x.shape[0]
    a = x.reshape(B, N1, N2, C)
    fp = jnp.concatenate([f, jnp.zeros_like(f)], axis=1).reshape(B, N1, N2, C)
    xr, xi = _fwd_fft_real(a)
    fr, fi = _fwd_fft_real(fp)
    pr = xr * fr - xi * fi
    pi = xr * fi + xi * fr
    out = _inv_fft_real(pr, pi)
    return out.reshape(B, L, C)


def _conv1d(x, w, b, stride=1):
    y = jax.lax.conv_general_dilated(
        x, w, window_strides=(stride,), padding='SAME',
        dimension_numbers=('NWC', 'WIO', 'NWC'))
    return jnp.tanh(y + b)


def _forward(x, params):
    B = x.shape[0]
    feature = x
    for (wt, bt), s in zip(params['enc'], (1, 1, 2)):
        feature = _conv1d(feature, wt, bt, s)
    inp = jnp.concatenate([jnp.ones((B, 1, C), x.dtype),
                           jnp.zeros((B, L - 1, C), x.dtype)], axis=1)
    filt = jnp.zeros((B, L // 2, 0), x.dtype)
    for i in range(4):
        f = jnp.concatenate([feature, filt], axis=2)
        for (wt, bt) in params['fgb'][i]:
            f = _conv1d(f, wt, bt)
        filt = f
        if i == 0:
            # circ_conv with the unit impulse is a zero-padded copy
            inp = jnp.concatenate([filt, jnp.zeros_like(filt)], axis=1)
        else:
            inp = _circ_conv_mm(inp, filt)
        wt, bt = params['cb'][i]
        inp = _conv1d(inp, wt, bt)
    out = inp
    for (wt, bt) in params['out']:
        out = _conv1d(out, wt, bt)
    return out


def kernel(x, params):
    x = np.asarray(x)
    B = x.shape[0]
    shard = B // N_CORES
    devs = jax.devices()[:N_CORES]

    fwd = jax.pmap(_forward, axis_name='b', in_axes=(0, None), devices=devs)
    xs = x.reshape(N_CORES, shard, *x.shape[1:])
    out = fwd(xs, params)
    out = np.asarray(out).reshape(B, *out.shape[2:])
    return out.astype(np.float32)
